# revision 2
# baseline (speedup 1.0000x reference)
"""Trainium2 Bass kernel for nn_CausalDerivative (per-node MLP stack).

Computation (reference):
    x = where(discrete_mask, (inputs > 0), inputs)          # straight-through gate
    W1m = W1 * M[:, None, :]   (M = adjacency, last row one-hot @ last col)
    h = relu(einsum('bn,ihn->bih', x, W1m))                 # [B, N, H]
    out = einsum('bih,ih->bi', h, W2)                       # [B, N]

Strategy: pure data-parallel over 8 NeuronCores (batch sharded 4096/core).
On each core, two chained matmuls on the PE in persistent 64x64 tiling mode
(4 concurrent subarray tiles), with the relu eviction of the intermediate
split between the Scalar (ACT) and Vector (DVE) engines:

  stage 1:  h^T[ih, b] = W1T[n, ih]^T @ x^T[n, b]     (K=64, 4x 64x64 tiles)
  evict:    relu PSUM -> SBUF [128, 1024] combined A+B tile, alternating
            between the ACT and DVE engines (the throughput bottleneck)
  stage 2:  out[i, b] += W2blk[ih, i]^T @ relu_h[ih, b]  (K=64 halves, accum)

W2 is folded into a block-diagonal [IH, N] matrix so the signed, segmented
reduction over each node's 64 hidden units happens inside the PE accumulation.
"""

import os
import numpy as np

import concourse.bass as bass
import concourse.tile as tile
from concourse import mybir, bacc
from concourse.bass import ts
from concourse.bass_utils import run_bass_kernel_spmd

B, N, H = 32768, 64, 64
IH = N * H                    # 4096 hidden units total
N_CORES = 8
BL = B // N_CORES             # 4096 batch rows per core
HALF = BL // 2                # 2048 (batch half per SBUF partition group)
BW = 512                      # batch tile width (PE moving free dim)
NPAIR = HALF // BW            # 4 batch pairs per core
NCHUNK = IH // 128            # 32 ih chunks of 128 units (2 nodes each)

F32 = mybir.dt.float32
F32R = mybir.dt.float32r
BF16 = mybir.dt.bfloat16
DT = BF16          # compute dtype for matmul operands
import ml_dtypes
NP_DT = ml_dtypes.bfloat16

# exec time of the last traced run (ns), for the test harness
LAST_EXEC_NS = None

_compiled = {}


def _build_module(n_disc: int):
    """Emit the per-core Bass module (same program for all 8 cores)."""
    nc = bacc.Bacc("TRN2", target_bir_lowering=False, debug=False)
    xt = nc.dram_tensor("xt", [N, BL], DT, kind="ExternalInput").ap()
    w1 = nc.dram_tensor("w1", [N, IH], DT, kind="ExternalInput").ap()
    w2 = nc.dram_tensor("w2", [128, NCHUNK * 64], DT, kind="ExternalInput").ap()
    out = nc.dram_tensor("out", [N, BL], F32, kind="ExternalOutput").ap()

    with tile.TileContext(nc) as tc:
        with (
            tc.tile_pool(name="consts", bufs=1) as consts,
            tc.tile_pool(name="ha", bufs=7) as hpa,
            tc.tile_pool(name="so", bufs=2) as sop,
            tc.tile_pool(name="ps_ab", bufs=3, space="PSUM") as ps_ab,
            tc.tile_pool(name="accs", bufs=2, space="PSUM") as accs,
        ):
            sx = consts.tile([128, HALF], DT)
            w1s = consts.tile([128, IH], DT)
            w2s = consts.tile([128, NCHUNK * 64], DT)

            # Startup loads: few large DMAs, triggers spread across engine
            # queues (trigger issue costs ~600ns each, serialized per queue).
            # x first (the gate op depends on it), weights in parallel.
            # first batch-pair slice lands first so the pipeline starts early
            nc.sync.dma_start(sx[0:64, 0:BW], xt[:, 0:BW])
            nc.gpsimd.dma_start(sx[64:128, 0:BW], xt[:, HALF : HALF + BW])
            nc.scalar.dma_start(w1s[0:64, 0 : IH // 2], w1[:, 0 : IH // 2])
            nc.sync.dma_start(sx[0:64, BW:HALF], xt[:, BW:HALF])
            nc.gpsimd.dma_start(sx[64:128, BW:HALF], xt[:, HALF + BW : BL])
            nc.scalar.dma_start(w1s[64:128, 0 : IH // 2], w1[:, 0 : IH // 2])
            nc.scalar.dma_start(w1s[0:64, IH // 2 : IH], w1[:, IH // 2 : IH])
            nc.scalar.dma_start(w1s[64:128, IH // 2 : IH], w1[:, IH // 2 : IH])
            nc.sync.dma_start(w2s[:, :], w2[:, :])

            sxr = sx[:]
            w1r = w1s[:]
            w2r = w2s[:]

            # Software-pipelined main loop: stage 2 consumption lags stage 1
            # by DELAY chunks so the in-order PE queue never blocks on a
            # pending relu eviction.
            DELAY = 4
            jtot = 0
            for p in range(NPAIR):
                acc1 = accs.tile([128, BW], F32, tag="acc")
                acc2 = accs.tile([128, BW], F32, tag="acc")
                bs = ts(p, BW)
                hAq = {}
                # gate this pair's batch columns (first n_disc features -> x>0)
                if n_disc > 0:
                    nc.vector.tensor_scalar(
                        sx[0:n_disc, bs], sx[0:n_disc, bs], 0.0, None,
                        op0=mybir.AluOpType.is_gt)
                    nc.vector.tensor_scalar(
                        sx[64 : 64 + n_disc, bs], sx[64 : 64 + n_disc, bs], 0.0,
                        None, op0=mybir.AluOpType.is_gt)
                for jj in range(NCHUNK + DELAY):
                    if jj < NCHUNK:
                        j = jj
                        # ---- stage 1: h^T chunk j, both batch halves into
                        # one combined 2-bank PSUM tile (A cols 0:BW from
                        # row-tiles T0/T2, B cols BW:2BW from T8/T10) ----
                        psAB = ps_ab.tile([128, 2 * BW], F32)
                        c0 = bass.ds(j * 128, 64)
                        c1 = bass.ds(j * 128 + 64, 64)
                        aslc = bass.ds(0, BW)
                        bslc = bass.ds(BW, BW)
                        nc.tensor.matmul(psAB[0:64, aslc], w1r[0:64, c0], sxr[0:64, bs])
                        nc.tensor.matmul(psAB[64:128, aslc], w1r[0:64, c1], sxr[0:64, bs])
                        nc.tensor.matmul(psAB[0:64, bslc], w1r[64:128, c0], sxr[64:128, bs])
                        nc.tensor.matmul(psAB[64:128, bslc], w1r[64:128, c1], sxr[64:128, bs])

                        # ---- relu eviction: one [128, 2*BW] op per chunk,
                        # alternating between ACT and DVE ----
                        hAB = hpa.tile([128, 2 * BW], DT)
                        if jtot % 2 == 0:
                            nc.scalar.activation(hAB[:], psAB[:], mybir.ActivationFunctionType.Relu)
                        else:
                            nc.vector.tensor_scalar_max(hAB[:], psAB[:], 0.0)
                        hAq[j] = hAB
                        jtot += 1

                    if jj >= DELAY:
                        j = jj - DELAY
                        # ---- stage 2: accumulate W2blk^T @ relu_h ----
                        st, sp = j == 0, j == NCHUNK - 1
                        ws = ts(j, 64)
                        hAB = hAq.pop(j)
                        hAr = hAB[:, bass.ds(0, BW)]
                        hBr = hAB[:, bass.ds(BW, BW)]
                        nc.tensor.matmul(acc1[0:64, :], w2r[0:64, ws], hAr[0:64, :],
                                         start=st, stop=sp, skip_group_check=True)
                        nc.tensor.matmul(acc2[0:64, :], w2r[64:128, ws], hAr[64:128, :],
                                         start=st, stop=sp, skip_group_check=True)
                        nc.tensor.matmul(acc1[64:128, :], w2r[0:64, ws], hBr[0:64, :],
                                         start=st, stop=sp, skip_group_check=True)
                        nc.tensor.matmul(acc2[64:128, :], w2r[64:128, ws], hBr[64:128, :],
                                         start=st, stop=sp, skip_group_check=True)

                # ---- combine the two K-half accumulators, store ----
                # (tensor_tensor may read at most one PSUM operand)
                t2 = sop.tile([128, BW], F32, tag="t2")
                nc.scalar.activation(t2[:], acc2[:], mybir.ActivationFunctionType.Copy)
                so = sop.tile([128, BW], F32)
                nc.vector.tensor_tensor(so[:], acc1[:], t2[:], op=mybir.AluOpType.add)
                nc.sync.dma_start(out[:, bass.ds(p * BW, BW)], so[0:64, :])
                nc.sync.dma_start(out[:, bass.ds(HALF + p * BW, BW)], so[64:128, :])

    nc.compile()
    return nc


def kernel(t, inputs, W1, W2, adjacency, discrete_mask, **_ignored):
    global LAST_EXEC_NS
    inputs = np.asarray(inputs, np.float32)
    W1 = np.asarray(W1, np.float32)
    W2 = np.asarray(W2, np.float32)
    adjacency = np.asarray(adjacency, np.float32)
    discrete_mask = np.asarray(discrete_mask)

    n_disc = int(discrete_mask.sum())
    # discrete features are a contiguous prefix in this model
    assert bool(np.all(discrete_mask[:n_disc])), "expect prefix discrete mask"

    # ---- host-side weight folding / layout ----
    M = adjacency.copy()
    one_hot_last = np.zeros(N, np.float32)
    one_hot_last[-1] = 1.0
    M[-1] = M[-1] * one_hot_last
    W1m = W1 * M[:, None, :]                      # [N, H, N]
    w1t = np.ascontiguousarray(W1m.reshape(IH, N).T)   # [N, IH]

    w2blk = np.zeros((IH, N), np.float32)
    w2blk[np.arange(IH), np.repeat(np.arange(N), H)] = W2.reshape(IH)
    # chunk-major: [128, NCHUNK*64], chunk j at cols [64j, 64j+64)
    w2s = np.ascontiguousarray(
        w2blk.reshape(NCHUNK, 128, 64).transpose(1, 0, 2).reshape(128, NCHUNK * 64)
    )

    xt = np.ascontiguousarray(inputs.T)           # [N, B]

    if n_disc not in _compiled:
        _compiled[n_disc] = _build_module(n_disc)
    nc = _compiled[n_disc]

    w1t_d = w1t.astype(NP_DT)
    w2s_d = w2s.astype(NP_DT)
    xt_d = xt.astype(NP_DT)
    in_maps = [
        {
            "xt": np.ascontiguousarray(xt_d[:, c * BL : (c + 1) * BL]),
            "w1": w1t_d,
            "w2": w2s_d,
        }
        for c in range(N_CORES)
    ]

    trace = bool(int(os.environ.get("KERNEL_TRACE", "0")))
    res = run_bass_kernel_spmd(
        nc, in_maps, core_ids=list(range(N_CORES)), trace=trace
    )
    if trace:
        LAST_EXEC_NS = res.exec_time_ns
        globals()["LAST_RESULT"] = res

    outT = np.concatenate([res.results[c]["out"] for c in range(N_CORES)], axis=1)
    return np.ascontiguousarray(outT.T)



# revision 4
# speedup vs baseline: 1.0444x; 1.0444x over previous
"""Trainium2 Bass kernel for nn_CausalDerivative (per-node MLP stack).

Computation (reference):
    x = where(discrete_mask, (inputs > 0), inputs)          # straight-through gate
    W1m = W1 * M[:, None, :]   (M = adjacency, last row one-hot @ last col)
    h = relu(einsum('bn,ihn->bih', x, W1m))                 # [B, N, H]
    out = einsum('bih,ih->bi', h, W2)                       # [B, N]

Strategy: pure data-parallel over 8 NeuronCores (batch sharded 4096/core).

Per core, the kernel is paced by the relu eviction of the 16.8M-element
intermediate (PSUM f32 -> SBUF bf16, 1 elem/cycle/lane on each of DVE and
ACT — the only two PSUM-capable engines).  Design:

  - |W2| is folded into W1 host-side (relu(|w|z) == |w|relu(z)), so the
    eviction is a plain relu and stage-2 weights are signs (+-1).
  - stage 1: per 128-unit ih chunk, 4 concurrent 64x64 quadrant matmuls
    produce z' [128, 1024] f32 (A|B batch halves) in one 2-bank PSUM tile.
  - eviction: chunks alternate DVE / ACT so both engines run at capacity.
  - stage 2: chunks are processed in pairs (2t, 2t+1); each chunk issues
    two K=128, M=32 matmuls into distinct 32-partition col strips of a
    single f32 accumulator bank (4 strips <- 4 concurrent streams), with
    PSUM-side accumulation across all 16 chunk pairs of a batch tile.
    This halves stage-2 PE time vs a 64-wide block-diagonal scheme and
    removes all accumulator-combine vector ops.
  - node rows come out in a stripe-permuted order; the host unpermutes.
"""

import os
import numpy as np

import concourse.bass as bass
import concourse.tile as tile
from concourse import mybir, bacc
from concourse.bass import ts
from concourse.bass_utils import run_bass_kernel_spmd

B, N, H = 32768, 64, 64
IH = N * H                    # 4096 hidden units total
N_CORES = 8
BL = B // N_CORES             # 4096 batch rows per core
HALF = BL // 2                # 2048 (batch half per SBUF partition group)
BW = 512                      # batch tile width (PE moving free dim)
NPAIR = HALF // BW            # 4 batch tiles per core
NCHUNK = IH // 128            # 32 ih chunks of 128 units (2 nodes each)
NK = NCHUNK // 2              # 16 chunk-pairs (k-tiles) per batch tile

F32 = mybir.dt.float32
BF16 = mybir.dt.bfloat16
DT = BF16
import ml_dtypes
NP_DT = ml_dtypes.bfloat16

LAST_EXEC_NS = None

_compiled = {}


def _build_module(n_disc: int):
    """Emit the per-core Bass module (same program for all 8 cores)."""
    nc = bacc.Bacc("TRN2", target_bir_lowering=False, debug=False)
    xt = nc.dram_tensor("xt", [N, BL], DT, kind="ExternalInput").ap()
    w1 = nc.dram_tensor("w1", [N, IH], DT, kind="ExternalInput").ap()
    w2 = nc.dram_tensor("w2", [128, NCHUNK * 32], DT, kind="ExternalInput").ap()
    out = nc.dram_tensor("out", [N, BL], F32, kind="ExternalOutput").ap()

    DELAY = 2                 # stage-2 lags stage-1 by DELAY k-tiles
    NKT = NPAIR * NK          # 64 k-tiles total

    with tile.TileContext(nc) as tc:
        with (
            tc.tile_pool(name="consts", bufs=1) as consts,
            tc.tile_pool(name="hp", bufs=8) as hp,
            tc.tile_pool(name="so", bufs=2) as sop,
            tc.tile_pool(name="ps", bufs=3, space="PSUM") as psp,
            tc.tile_pool(name="accs", bufs=2, space="PSUM") as accs,
        ):
            sx = consts.tile([128, HALF], DT)
            w1s = consts.tile([128, IH], DT)
            w2s = consts.tile([128, NCHUNK * 32], DT)

            # Startup loads. First-needed slices first: x tile-0 columns,
            # then w1 front chunks; weight traffic stays off ACT/DVE queues.
            nc.sync.dma_start(sx[0:64, 0:BW], xt[:, 0:BW])
            nc.gpsimd.dma_start(sx[64:128, 0:BW], xt[:, HALF : HALF + BW])
            nc.gpsimd.dma_start(w1s[0:64, 0 : IH // 4], w1[:, 0 : IH // 4])
            nc.gpsimd.dma_start(w1s[64:128, 0 : IH // 4], w1[:, 0 : IH // 4])
            nc.sync.dma_start(sx[0:64, BW:HALF], xt[:, BW:HALF])
            nc.sync.dma_start(sx[64:128, BW:HALF], xt[:, HALF + BW : BL])
            nc.sync.dma_start(w2s[:, :], w2[:, :])
            nc.gpsimd.dma_start(w1s[0:64, IH // 4 : IH], w1[:, IH // 4 : IH])
            nc.gpsimd.dma_start(w1s[64:128, IH // 4 : IH], w1[:, IH // 4 : IH])

            hq = {}
            accq = {}

            def stage1(kt):
                p, t = divmod(kt, NK)
                bs = ts(p, BW)
                if t == 0 and n_disc > 0:
                    # straight-through gate for this batch tile's columns
                    nc.vector.tensor_scalar(
                        sx[0:n_disc, bs], sx[0:n_disc, bs], 0.0, None,
                        op0=mybir.AluOpType.is_gt)
                    nc.vector.tensor_scalar(
                        sx[64 : 64 + n_disc, bs], sx[64 : 64 + n_disc, bs], 0.0,
                        None, op0=mybir.AluOpType.is_gt)
                for u in range(2):          # the two chunks of this k-tile
                    j = 2 * t + u
                    ps = psp.tile([128, 2 * BW], F32)
                    c0 = bass.ds(j * 128, 64)
                    c1 = bass.ds(j * 128 + 64, 64)
                    asl = bass.ds(0, BW)
                    bsl = bass.ds(BW, BW)
                    nc.tensor.matmul(ps[0:64, asl], w1s[0:64, c0], sx[0:64, bs])
                    nc.tensor.matmul(ps[64:128, asl], w1s[0:64, c1], sx[0:64, bs])
                    nc.tensor.matmul(ps[0:64, bsl], w1s[64:128, c0], sx[64:128, bs])
                    nc.tensor.matmul(ps[64:128, bsl], w1s[64:128, c1], sx[64:128, bs])
                    # eviction: relu PSUM f32 -> SBUF bf16, DVE/ACT alternating
                    h = hp.tile([128, 2 * BW], DT)
                    if u == 0:
                        nc.vector.tensor_scalar_max(h[:], ps[:], 0.0)
                    else:
                        nc.scalar.activation(h[:], ps[:],
                                             mybir.ActivationFunctionType.Relu)
                    hq[j] = h

            def stage2(kt):
                p, t = divmod(kt, NK)
                if t == 0:
                    accq[p] = accs.tile([128, BW], F32, name="acc", tag="acc")
                acc = accq[p]
                st, sp = t == 0, t == NK - 1
                hA = hq.pop(2 * t)
                hB = hq.pop(2 * t + 1)
                wA = w2s[:, bass.ds(32 * (2 * t), 32)]
                wB = w2s[:, bass.ds(32 * (2 * t + 1), 32)]
                asl = bass.ds(0, BW)
                bsl = bass.ds(BW, BW)
                # 4 concurrent K=128, M=32 matmuls into distinct col strips
                nc.tensor.matmul(acc[0:32, :], wA, hA[:, asl], start=st, stop=sp,
                                 skip_group_check=True, tile_position=(0, 0))
                nc.tensor.matmul(acc[64:96, :], wA, hA[:, bsl], start=st, stop=sp,
                                 skip_group_check=True, tile_position=(0, 64))
                nc.tensor.matmul(acc[32:64, :], wB, hB[:, asl], start=st, stop=sp,
                                 skip_group_check=True, tile_position=(0, 32))
                nc.tensor.matmul(acc[96:128, :], wB, hB[:, bsl], start=st, stop=sp,
                                 skip_group_check=True, tile_position=(0, 96))
                if sp:
                    acc = accq.pop(p)
                    so = sop.tile([128, BW], F32)
                    nc.scalar.activation(so[:], acc[:],
                                         mybir.ActivationFunctionType.Copy)
                    nc.sync.dma_start(out[:, bass.ds(p * BW, BW)], so[0:64, :])
                    nc.sync.dma_start(out[:, bass.ds(HALF + p * BW, BW)],
                                      so[64:128, :])

            for kt in range(NKT + DELAY):
                if kt < NKT:
                    stage1(kt)
                if kt >= DELAY:
                    stage2(kt - DELAY)

    nc.compile()
    return nc


# dram-out row r holds node PERM[r] (stripe-packed stage-2 layout)
PERM = np.array([4 * ((p % 32) // 2) + 2 * (p // 32) + (p % 2)
                 for p in range(64)])


def kernel(t, inputs, W1, W2, adjacency, discrete_mask, **_ignored):
    global LAST_EXEC_NS
    inputs = np.asarray(inputs, np.float32)
    W1 = np.asarray(W1, np.float32)
    W2 = np.asarray(W2, np.float32)
    adjacency = np.asarray(adjacency, np.float32)
    discrete_mask = np.asarray(discrete_mask)

    n_disc = int(discrete_mask.sum())
    assert bool(np.all(discrete_mask[:n_disc])), "expect prefix discrete mask"

    # ---- host-side weight folding / layout ----
    M = adjacency.copy()
    one_hot_last = np.zeros(N, np.float32)
    one_hot_last[-1] = 1.0
    M[-1] = M[-1] * one_hot_last
    W1m = W1 * M[:, None, :]                      # [N, H, N]
    # fold |W2| into W1 rows: relu(|w| z) == |w| relu(z); signs go to stage 2
    W1e = W1m * np.abs(W2)[:, :, None]
    w1t = np.ascontiguousarray(W1e.reshape(IH, N).T)   # [N, IH]

    sgn = np.sign(W2).astype(np.float32)          # [N, H]
    w2s = np.zeros((128, NCHUNK * 32), np.float32)
    for j in range(NCHUNK):
        for u in range(2):
            node = 2 * j + u
            m = 2 * (j // 2) + u
            w2s[64 * u : 64 * u + 64, 32 * j + m] = sgn[node]

    xt = np.ascontiguousarray(inputs.T)           # [N, B]

    if n_disc not in _compiled:
        _compiled[n_disc] = _build_module(n_disc)
    nc = _compiled[n_disc]

    w1t_d = w1t.astype(NP_DT)
    w2s_d = w2s.astype(NP_DT)
    xt_d = xt.astype(NP_DT)
    in_maps = [
        {
            "xt": np.ascontiguousarray(xt_d[:, c * BL : (c + 1) * BL]),
            "w1": w1t_d,
            "w2": w2s_d,
        }
        for c in range(N_CORES)
    ]

    trace = bool(int(os.environ.get("KERNEL_TRACE", "0")))
    res = run_bass_kernel_spmd(
        nc, in_maps, core_ids=list(range(N_CORES)), trace=trace
    )
    if trace:
        LAST_EXEC_NS = res.exec_time_ns
        globals()["LAST_RESULT"] = res

    outT = np.concatenate([res.results[c]["out"] for c in range(N_CORES)], axis=1)
    # rows are stripe-permuted: row r holds node PERM[r]
    unperm = np.empty_like(outT)
    unperm[PERM] = outT
    return np.ascontiguousarray(unperm.T)
